# revision 3
# baseline (speedup 1.0000x reference)
"""Trainium2 Bass kernel for quantized-MoE Bottleneck (nn_Bottleneck_37503654429269).

v4 layout (from v3 + trace-driven rebalance):
- bf16 integer matmuls; exact round via +-2^23 fp32 trick.
- Host-side expert routing: (3,1) or (2,2) sample groups per core.
- 3-way elementwise split: ACT does Relu-affine (psum drains + x scale),
  DVE does the single round op (+RB,-RB -> bf16), GpSimd does min-clamps.
- GN tail: gng folded into conv3 drain scale; per-(stats-group,sample)
  scalars broadcast across partitions with one K=1 ones-matmul into PSUM;
  tail is stt (S3*rc + x) on DVE + relu-bias ts on GpSimd/DVE; bf16 out.
- Packed per-group weight dram tensors, prioritized DMA order, SQRT prewarm.
"""

import numpy as np

BITS = (2, 4, 8)
EPS = 1e-5
B, C_IN, H, W = 32, 1024, 14, 14
WIDTH, OUTC = 256, 1024
PIX = H * W  # 196
NCORES = 8
RB = float(2.0 ** 23)

_NC_CACHE = {}


# ----------------------------------------------------------------------------
# Device program
# ----------------------------------------------------------------------------

def _build_nc(group_sizes):
    from contextlib import ExitStack
    import concourse.bacc as bacc
    import concourse.mybir as mybir
    import concourse.tile as tile

    F32 = mybir.dt.float32
    BF16 = mybir.dt.bfloat16
    ALU = mybir.AluOpType
    ACT = mybir.ActivationFunctionType

    NG = len(group_sizes)
    NS = sum(group_sizes)
    assert NS == 4
    slot0 = [sum(group_sizes[:g]) for g in range(NG)]
    # chunks of <=2 samples (local indices within group)
    chunks = {g: [(i, min(2, group_sizes[g] - i))
                  for i in range(0, group_sizes[g], 2)] for g in range(NG)}

    nc = bacc.Bacc("TRN2", target_bir_lowering=False, debug=False,
                   num_devices=NCORES)

    # ---- dram tensors
    x_d = nc.dram_tensor("x", [128, 8, 4 * PIX], F32, kind="ExternalInput")
    # packed weights per group: w1 [8,256] | w2 [9,2,256] | w3 [2,1024]
    WCOL = 8 * 256 + 9 * 2 * 256 + 2 * 1024  # 8704
    wp_d = nc.dram_tensor("wp", [NG, 128, WCOL], BF16, kind="ExternalInput")
    # per-partition consts:
    # XS[NG] XB[NG] A1[2NG] B1[2NG] A2[2NG] B2[2NG] GNB[8] GNG[8]
    NCC = 10 * NG + 16
    cc_d = nc.dram_tensor("cc", [128, NCC], F32, kind="ExternalInput")
    # row consts: per group c3e[4ns], c3sq[4ns]
    GRN = sum(8 * n for n in group_sizes)
    gr_d = nc.dram_tensor("gr", [1, GRN], F32, kind="ExternalInput")
    out_d = nc.dram_tensor("out", [128, 8, 4 * PIX], BF16,
                           kind="ExternalOutput")

    with tile.TileContext(nc) as tc, ExitStack() as ctx:
        res = ctx.enter_context(tc.tile_pool(name="res", bufs=1))
        rot = ctx.enter_context(tc.tile_pool(name="rot", bufs=4))
        srt = ctx.enter_context(tc.tile_pool(name="srt", bufs=2))
        mmp = ctx.enter_context(tc.tile_pool(name="mmp", bufs=6, space="PSUM"))
        smp = ctx.enter_context(tc.tile_pool(name="smp", bufs=2, space="PSUM"))

        # ---- small consts + ACT table prewarm (Sqrt set) before DMAs land
        ONES = res.tile([128, 1], F32, name="ONES", tag="ONES")
        nc.vector.memset(ONES, 1.0)
        ONE1 = res.tile([1, 128], F32, name="ONE1", tag="ONE1")
        nc.vector.memset(ONE1, 1.0)
        WRM = res.tile([1, 2], F32, name="WRM", tag="WRM")
        nc.scalar.activation(out=WRM[:, 0:1], in_=ONE1[:, 0:1], func=ACT.Sqrt,
                             bias=0.0, scale=1.0)
        nc.scalar.activation(out=WRM[:, 1:2], in_=ONE1[:, 0:1], func=ACT.Relu,
                             bias=0.0, scale=1.0)

        # ---- input tiles
        CC = res.tile([128, NCC], F32, name="CC", tag="CC")
        o = 0
        XS = CC[:, o:o + NG]; o += NG
        XB = CC[:, o:o + NG]; o += NG
        A1 = CC[:, o:o + 2 * NG].rearrange("p (m g) -> p m g", m=2); o += 2 * NG
        B1 = CC[:, o:o + 2 * NG].rearrange("p (m g) -> p m g", m=2); o += 2 * NG
        A2 = CC[:, o:o + 2 * NG].rearrange("p (m g) -> p m g", m=2); o += 2 * NG
        B2 = CC[:, o:o + 2 * NG].rearrange("p (m g) -> p m g", m=2); o += 2 * NG
        GNB = CC[:, o:o + 8]; o += 8
        GNG = CC[:, o:o + 8]; o += 8

        GR = res.tile([1, GRN], F32, name="GR", tag="GR")

        XT = res.tile([128, 8, 4 * PIX], F32, name="XT", tag="XT")
        WP = [res.tile([128, WCOL], BF16, name=f"WP{g}", tag=f"WP{g}")
              for g in range(NG)]
        W1 = [WP[g][:, 0:2048].rearrange("p (k m) -> p k m", k=8)
              for g in range(NG)]
        W2 = [WP[g][:, 2048:6656].rearrange("p (t k m) -> p t k m", t=9, k=2)
              for g in range(NG)]
        W3 = [WP[g][:, 6656:8704].rearrange("p (k m) -> p k m", k=2)
              for g in range(NG)]

        # ---- DMA issue order (priority): cc, x0, w1g0, x1, w23g0, x2,
        #      w1g1.., x3, w23g1.., gr
        nc.sync.dma_start(out=CC, in_=cc_d.ap())
        nc.sync.dma_start(out=XT[:, 0:2, :], in_=x_d.ap()[:, 0:2, :])
        nc.sync.dma_start(out=WP[0][:, 0:2048], in_=wp_d.ap()[0, :, 0:2048])
        nc.sync.dma_start(out=XT[:, 2:4, :], in_=x_d.ap()[:, 2:4, :])
        nc.sync.dma_start(out=WP[0][:, 2048:], in_=wp_d.ap()[0, :, 2048:])
        nc.sync.dma_start(out=XT[:, 4:6, :], in_=x_d.ap()[:, 4:6, :])
        if NG > 1:
            nc.sync.dma_start(out=WP[1][:, 0:2048],
                              in_=wp_d.ap()[1, :, 0:2048])
        nc.sync.dma_start(out=XT[:, 6:8, :], in_=x_d.ap()[:, 6:8, :])
        if NG > 1:
            nc.sync.dma_start(out=WP[1][:, 2048:], in_=wp_d.ap()[1, :, 2048:])
        nc.sync.dma_start(out=GR, in_=gr_d.ap())

        # ---- persistent stage tiles
        Xq = [res.tile([128, 8, group_sizes[g] * PIX], BF16,
                       name=f"Xq{g}", tag=f"Xq{g}") for g in range(NG)]
        HP = [res.tile([128, 2, group_sizes[g], 16, 18], BF16,
                       name=f"HP{g}", tag=f"HP{g}") for g in range(NG)]
        Q2 = [res.tile([128, 2, group_sizes[g] * PIX], BF16,
                       name=f"Q2{g}", tag=f"Q2{g}") for g in range(NG)]
        S3 = [res.tile([128, 8, group_sizes[g] * PIX], BF16,
                       name=f"S3{g}", tag=f"S3{g}") for g in range(NG)]
        OT = [res.tile([128, 8, group_sizes[g] * PIX], BF16,
                       name=f"OT{g}", tag=f"OT{g}") for g in range(NG)]
        BST = [res.tile([128, 8 * group_sizes[g] * 8], F32,
                        name=f"BST{g}", tag=f"BST{g}") for g in range(NG)]
        QQ = [res.tile([128, 8, group_sizes[g]], F32,
                       name=f"QQ{g}", tag=f"QQ{g}") for g in range(NG)]
        FR = [res.tile([1, 2, 4 * group_sizes[g]], F32,
                       name=f"FR{g}", tag=f"FR{g}") for g in range(NG)]

        for g in range(NG):
            nc.gpsimd.memset(HP[g], 0.0)

        # ---------------- x quantization ----------------
        # q = min(round(relu(x)*(lv-1)), lv-1) in bf16
        for kt in range(8):
            for g in range(NG):
                ns = group_sizes[g]
                xcols = XT[:, kt, slot0[g] * PIX:(slot0[g] + ns) * PIX]
                u = rot.tile([128, ns * PIX], F32, name="xu", tag=f"xu{g}")
                nc.scalar.activation(out=u, in_=xcols, func=ACT.Relu,
                                     bias=0.0, scale=XS[:, g:g + 1])
                xq = Xq[g][:, kt, :]
                nc.vector.tensor_scalar(out=xq, in0=u, scalar1=RB, scalar2=RB,
                                        op0=ALU.add, op1=ALU.subtract)
                nc.gpsimd.tensor_scalar(out=xq, in0=xq,
                                        scalar1=XB[:, g:g + 1], scalar2=None,
                                        op0=ALU.min)

        # ---------------- conv1 + bn1 + quant ----------------
        def c1_block(g, mo, c0, ncnk):
            ps = mmp.tile([128, ncnk * PIX], F32, name="c1ps", tag="mm")
            for kt in range(8):
                nc.tensor.matmul(
                    ps, W1[g][:, kt, mo * 128:(mo + 1) * 128],
                    Xq[g][:, kt, c0 * PIX:(c0 + ncnk) * PIX],
                    start=(kt == 0), stop=(kt == 7))
            t1 = rot.tile([128, ncnk * PIX], F32, name="t1", tag="t1")
            nc.scalar.activation(out=t1, in_=ps, func=ACT.Relu,
                                 bias=B1[:, mo, g:g + 1],
                                 scale=A1[:, mo, g:g + 1])
            hview = HP[g][:, mo, c0:c0 + ncnk, 1:15, 2:16]
            nc.vector.tensor_scalar(
                out=hview,
                in0=t1.rearrange("p (s y x) -> p s y x", s=ncnk, y=14),
                scalar1=RB, scalar2=RB, op0=ALU.add, op1=ALU.subtract)
            nc.gpsimd.tensor_scalar(out=hview, in0=hview,
                                    scalar1=XB[:, g:g + 1], scalar2=None,
                                    op0=ALU.min)

        for g in range(NG):
            for c0, ncnk in chunks[g]:
                for mo in range(2):
                    c1_block(g, mo, c0, ncnk)

        # ---------------- conv2 + bn2 + quant ----------------
        def c2_block(g, mo, c0, ncnk):
            ps = mmp.tile([128, ncnk, 14, 14], F32, name="c2ps", tag="mm")
            first = True
            for ti, (dy, dx) in enumerate(
                    (dy, dx) for dy in range(3) for dx in range(3)):
                for kt in range(2):
                    nc.tensor.matmul(
                        ps, W2[g][:, ti, kt, mo * 128:(mo + 1) * 128],
                        HP[g][:, kt, c0:c0 + ncnk, dy:dy + 14, dx + 1:dx + 15],
                        start=first, stop=(ti == 8 and kt == 1))
                    first = False
            t2 = rot.tile([128, ncnk * PIX], F32, name="t2", tag="t2")
            nc.scalar.activation(out=t2,
                                 in_=ps.rearrange("p s y x -> p (s y x)"),
                                 func=ACT.Relu, bias=B2[:, mo, g:g + 1],
                                 scale=A2[:, mo, g:g + 1])
            qv = Q2[g][:, mo, c0 * PIX:(c0 + ncnk) * PIX]
            nc.vector.tensor_scalar(out=qv, in0=t2, scalar1=RB, scalar2=RB,
                                    op0=ALU.add, op1=ALU.subtract)
            nc.gpsimd.tensor_scalar(out=qv, in0=qv, scalar1=XB[:, g:g + 1],
                                    scalar2=None, op0=ALU.min)

        # ---------------- conv3 + stats ----------------
        def c3_block(g, mo, c0, ncnk):
            ns = group_sizes[g]
            ps = mmp.tile([128, ncnk * PIX], F32, name="c3ps", tag="mm")
            for kt in range(2):
                nc.tensor.matmul(
                    ps, W3[g][:, kt, mo * 128:(mo + 1) * 128],
                    Q2[g][:, kt, c0 * PIX:(c0 + ncnk) * PIX],
                    start=(kt == 0), stop=(kt == 1))
            bstv = BST[g][:, 0:8 * ns * 6].rearrange("p (t c) -> p t c", c=6)
            for si in range(ncnk):
                nc.vector.bn_stats(
                    out=bstv[:, mo * ns + c0 + si:mo * ns + c0 + si + 1, :],
                    in_=ps[:, si * PIX:(si + 1) * PIX])
            nc.scalar.activation(out=S3[g][:, mo, c0 * PIX:(c0 + ncnk) * PIX],
                                 in_=ps, func=ACT.Identity, bias=0.0,
                                 scale=GNG[:, mo:mo + 1])

        def stats_reduce(g):
            """msq + partition-reduce matmul; returns psum tile of sums."""
            ns = group_sizes[g]
            nst = 8 * ns
            mvi = BST[g][:, 0:nst * 6].rearrange(
                "p (t h c) -> p t h c", h=2, c=3)[:, :, :, 1]
            msq = BST[g][:, nst * 6:nst * 8].rearrange("p (t h) -> p t h", h=2)
            nc.vector.tensor_tensor(out=msq, in0=mvi, in1=mvi, op=ALU.mult)
            red = mmp.tile([1, nst * 8], F32, name="red", tag="mm")
            nc.tensor.matmul(red, ONES, BST[g], start=True, stop=True)
            return red

        def stats_chain(g, red):
            """red psum -> rcB psum [128, 2, 4ns] (R row 0, M row 1)."""
            ns = group_sizes[g]
            nst = 8 * ns
            nsc = 4 * ns
            Tg = srt.tile([1, nst * 8], F32, name="Tg", tag="Tg")
            nc.scalar.activation(out=Tg, in_=red, func=ACT.Copy,
                                 bias=0.0, scale=1.0)
            tv = Tg[:, 0:nst * 6].rearrange("p (m o s c) -> p m o s c",
                                            m=4, o=2, c=6)
            mv = Tg[:, nst * 6:nst * 8].rearrange("p (m o s c) -> p m o s c",
                                                  m=4, o=2, c=2)
            TB = srt.tile([1, nsc * 8], F32, name="TB", tag="TB")
            tb6 = TB[:, 0:nsc * 6].rearrange("p (m s c) -> p m s c", m=4, c=6)
            tbq = TB[:, nsc * 6:nsc * 8].rearrange("p (m s c) -> p m s c",
                                                   m=4, c=2)
            nc.vector.tensor_tensor(out=tb6, in0=tv[:, :, 0, :, :],
                                    in1=tv[:, :, 1, :, :], op=ALU.add)
            nc.vector.tensor_tensor(out=tbq, in0=mv[:, :, 0, :, :],
                                    in1=mv[:, :, 1, :, :], op=ALU.add)
            SC = srt.tile([1, 3 * nsc], F32, name="SC", tag="SC")
            scv = SC.rearrange("p (c t) -> p c t", c=3)
            nc.vector.tensor_tensor(out=scv[:, 0, :], in0=tb6[:, :, :, 1],
                                    in1=tb6[:, :, :, 4], op=ALU.add)
            nc.vector.tensor_tensor(out=scv[:, 1, :], in0=tb6[:, :, :, 2],
                                    in1=tb6[:, :, :, 5], op=ALU.add)
            nc.vector.tensor_tensor(out=scv[:, 2, :], in0=tbq[:, :, :, 0],
                                    in1=tbq[:, :, :, 1], op=ALU.add)
            MEAN = srt.tile([1, nsc], F32, name="MEAN", tag="MEAN")
            nc.vector.tensor_scalar(out=MEAN, in0=scv[:, 0, :],
                                    scalar1=1.0 / 512, scalar2=None,
                                    op0=ALU.mult)
            E2 = srt.tile([1, nsc], F32, name="E2", tag="E2")
            nc.vector.scalar_tensor_tensor(out=E2, in0=scv[:, 2, :],
                                           scalar=98.0, in1=scv[:, 1, :],
                                           op0=ALU.mult, op1=ALU.add)
            cbase = sum(8 * n for n in group_sizes[:g])
            C3E = GR[:, cbase:cbase + nsc]
            C3S = GR[:, cbase + nsc:cbase + 2 * nsc]
            VAR = srt.tile([1, nsc], F32, name="VAR", tag="VAR")
            nc.vector.tensor_scalar(out=E2, in0=E2,
                                    scalar1=1.0 / (2 * 128 * PIX),
                                    scalar2=None, op0=ALU.mult)
            nc.vector.tensor_tensor(out=VAR, in0=MEAN, in1=MEAN, op=ALU.mult)
            nc.vector.tensor_tensor(out=VAR, in0=E2, in1=VAR, op=ALU.subtract)
            nc.vector.tensor_tensor(out=VAR, in0=VAR, in1=C3S, op=ALU.mult)
            nc.vector.tensor_scalar(out=VAR, in0=VAR, scalar1=EPS,
                                    scalar2=None, op0=ALU.add)
            SD = srt.tile([1, nsc], F32, name="SD", tag="SD")
            nc.scalar.activation(out=SD, in_=VAR, func=ACT.Sqrt,
                                 bias=0.0, scale=1.0)
            RC = srt.tile([1, nsc], F32, name="RC", tag="RC")
            nc.vector.reciprocal(out=RC, in_=SD)
            nc.vector.tensor_tensor(out=FR[g][:, 0, :], in0=RC, in1=C3E,
                                    op=ALU.mult)
            nc.vector.scalar_tensor_tensor(out=FR[g][:, 1, :], in0=MEAN,
                                           scalar=-1.0, in1=FR[g][:, 0, :],
                                           op0=ALU.mult, op1=ALU.mult)
            rcb = smp.tile([128, 2, nsc], F32, name="rcb", tag="rcb")
            nc.tensor.matmul(rcb.rearrange("p a b -> p (a b)"), ONE1,
                             FR[g].rearrange("p a b -> p (a b)"),
                             start=True, stop=True)
            return rcb

        def qq_ops(g, rcb):
            ns = group_sizes[g]
            for mo in range(8):
                G3 = mo // 2
                nc.vector.tensor_scalar(
                    out=QQ[g][:, mo, :],
                    in0=rcb[:, 1, G3 * ns:(G3 + 1) * ns],
                    scalar1=GNG[:, mo:mo + 1], scalar2=GNB[:, mo:mo + 1],
                    op0=ALU.mult, op1=ALU.add)

        def tail_pair(g, rcb, mo, s, relu_on_gp):
            ns = group_sizes[g]
            G3 = mo // 2
            slot = slot0[g] + s
            V = rot.tile([128, PIX], F32, name="V", tag="V")
            nc.vector.scalar_tensor_tensor(
                out=V, in0=S3[g][:, mo, s * PIX:(s + 1) * PIX],
                scalar=rcb[:, 0, G3 * ns + s:G3 * ns + s + 1],
                in1=XT[:, mo, slot * PIX:(slot + 1) * PIX],
                op0=ALU.mult, op1=ALU.add)
            eng = nc.gpsimd if relu_on_gp else nc.vector
            eng.tensor_scalar(out=OT[g][:, mo, s * PIX:(s + 1) * PIX],
                              in0=V, scalar1=QQ[g][:, mo, s:s + 1],
                              scalar2=0.0, op0=ALU.add, op1=ALU.max)

        def out_dma(g, mo0, mo1):
            ns = group_sizes[g]
            nc.sync.dma_start(
                out=out_d.ap()[:, mo0:mo1,
                               slot0[g] * PIX:(slot0[g] + ns) * PIX],
                in_=OT[g][:, mo0:mo1, :])

        # ---------------- schedule ----------------
        glast = NG - 1
        # conv2+conv3 for g0
        for c0, ncnk in chunks[0]:
            for mo in range(2):
                c2_block(0, mo, c0, ncnk)
        for mo in range(8):
            for c0, ncnk in chunks[0]:
                c3_block(0, mo, c0, ncnk)
        red0 = stats_reduce(0)

        if NG > 1:
            # first conv2 psum-group of g1 keeps PE busy while g0 chain runs
            c2_block(1, 0, chunks[1][0][0], chunks[1][0][1])
        rcb0 = stats_chain(0, red0)
        qq_ops(0, rcb0)
        if NG > 1:
            for c0, ncnk in chunks[1]:
                for mo in range(2):
                    if mo == 0 and c0 == chunks[1][0][0]:
                        continue
                    c2_block(1, mo, c0, ncnk)
            for mo in range(8):
                for c0, ncnk in chunks[1]:
                    c3_block(1, mo, c0, ncnk)
            red1 = stats_reduce(1)

            # g0 tail overlaps g1 stats; relu on gpsimd to spare DVE
            for mo in range(8):
                for s in range(group_sizes[0]):
                    tail_pair(0, rcb0, mo, s, relu_on_gp=True)
                if mo == 3:
                    out_dma(0, 0, 4)
            out_dma(0, 4, 8)

            rcb1 = stats_chain(1, red1)
            qq_ops(1, rcb1)
            for mo in range(8):
                for s in range(group_sizes[1]):
                    tail_pair(1, rcb1, mo, s, relu_on_gp=False)
                if mo == 3:
                    out_dma(1, 0, 4)
            out_dma(1, 4, 8)
        else:
            for mo in range(8):
                for s in range(group_sizes[0]):
                    tail_pair(0, rcb0, mo, s, relu_on_gp=(s % 2 == 0))
                if mo == 3:
                    out_dma(0, 0, 4)
            out_dma(0, 4, 8)

    nc.compile()
    return nc


# ----------------------------------------------------------------------------
# Host side
# ----------------------------------------------------------------------------

def _quant_w(w, lv):
    n = max(lv // 2 - 1, 1)
    s = np.float32(np.abs(w).max()) + np.float32(1e-12)
    k = np.round((w.astype(np.float32) / s) * np.float32(n)).astype(np.float32)
    return k, np.float32(s) / np.float32(n)


def _assign_groups(mask):
    mask = np.asarray(mask).astype(np.int64)
    ids = {e: [int(i) for i in np.nonzero(mask == e)[0]] for e in range(3)}
    counts = [len(ids[e]) for e in range(3)]
    if all(c % 2 == 0 for c in counts):
        group_sizes = (2, 2)
        chunks2 = []
        for e in range(3):
            for j in range(0, counts[e], 2):
                chunks2.append((e, ids[e][j:j + 2]))
        assert len(chunks2) == 16
        core_samples = []
        core_experts = []
        for c in range(8):
            (ea, sa), (eb, sb) = chunks2[2 * c], chunks2[2 * c + 1]
            core_samples.append(sa + sb)
            core_experts.append([ea, eb])
        return group_sizes, core_samples, core_experts

    base = [c % 3 for c in counts]
    need = (8 - sum(base)) // 3
    t = [0, 0, 0]
    for e in range(3):
        cap = (counts[e] - base[e]) // 3
        take = min(cap, need)
        t[e] = take
        need -= take
        if need == 0:
            break
    assert need == 0
    b = [base[e] + 3 * t[e] for e in range(3)]
    a = [(counts[e] - b[e]) // 3 for e in range(3)]
    assert sum(a) == 8 and sum(b) == 8
    trip = []
    single = []
    for e in range(3):
        pos = 0
        for _ in range(a[e]):
            trip.append((e, ids[e][pos:pos + 3]))
            pos += 3
        for _ in range(b[e]):
            single.append((e, [ids[e][pos]]))
            pos += 1
        assert pos == counts[e]
    core_samples = []
    core_experts = []
    for c in range(8):
        ea, sa = trip[c]
        eb, sb = single[c]
        core_samples.append(sa + sb)
        core_experts.append([ea, eb])
    return (3, 1), core_samples, core_experts


def kernel(x, mask, w1, w2, w3, bn1_g, bn1_b, bn1_m, bn1_v,
           bn2_g, bn2_b, bn2_m, bn2_v, gn_g, gn_b):
    import ml_dtypes
    from concourse.bass_utils import run_bass_kernel_spmd

    bf16 = ml_dtypes.bfloat16
    f32 = np.float32
    x = np.asarray(x, f32)
    mask = np.asarray(mask)
    w1 = np.asarray(w1, f32)
    w2 = np.asarray(w2, f32)
    w3 = np.asarray(w3, f32)
    bn1 = [np.asarray(v, f32) for v in (bn1_g, bn1_b, bn1_m, bn1_v)]
    bn2 = [np.asarray(v, f32) for v in (bn2_g, bn2_b, bn2_m, bn2_v)]
    gn_g = np.asarray(gn_g, f32)
    gn_b = np.asarray(gn_b, f32)

    group_sizes, core_samples, core_experts = _assign_groups(mask)
    NG = len(group_sizes)

    lv_of = [2 ** b for b in BITS]
    K1, K2, K3 = {}, {}, {}
    CW = {}
    for e in range(3):
        lv = lv_of[e]
        k1, c1 = _quant_w(w1, lv)
        k2, c2 = _quant_w(w2, lv)
        k3, c3 = _quant_w(w3, lv)
        K1[e] = k1.reshape(256, 1024)
        K2[e] = k2.reshape(256, 256, 3, 3)
        K3[e] = k3.reshape(1024, 256)
        CW[e] = (c1, c2, c3)

    inv1 = bn1[0] / np.sqrt(bn1[3] + f32(EPS))
    bb1 = bn1[1] - bn1[2] * inv1
    inv2 = bn2[0] / np.sqrt(bn2[3] + f32(EPS))
    bb2 = bn2[1] - bn2[2] * inv2

    def pack_w(e):
        k1t = K1[e].T.reshape(8, 128, 256).transpose(1, 0, 2)
        k2t = K2[e].transpose(2, 3, 1, 0).reshape(9, 2, 128, 256)
        k2t = k2t.transpose(2, 0, 1, 3)
        k3t = K3[e].T.reshape(2, 128, 1024).transpose(1, 0, 2)
        return np.concatenate([
            np.ascontiguousarray(k1t).reshape(128, 2048),
            np.ascontiguousarray(k2t).reshape(128, 4608),
            np.ascontiguousarray(k3t).reshape(128, 2048)], axis=1).astype(bf16)

    packed = {e: pack_w(e) for e in set(int(v) for v in np.asarray(mask))}

    in_maps = []
    for c in range(8):
        sids = core_samples[c]
        experts = core_experts[c]

        xc = x[sids].reshape(4, 8, 128, PIX).transpose(2, 1, 0, 3) \
                    .reshape(128, 8, 4 * PIX).copy()
        wpc = np.stack([packed[experts[g]] for g in range(NG)])

        glv = [lv_of[experts[g]] for g in range(NG)]
        cc = np.zeros((128, 10 * NG + 16), f32)
        cc[:, 0:NG] = [lv - 1 for lv in glv]          # xs
        cc[:, NG:2 * NG] = [lv - 1 for lv in glv]     # xb
        a1 = np.zeros((128, 2, NG), f32)
        b1 = np.zeros((128, 2, NG), f32)
        a2 = np.zeros((128, 2, NG), f32)
        b2 = np.zeros((128, 2, NG), f32)
        for g in range(NG):
            e = experts[g]
            lv = glv[g]
            c1, c2, c3 = CW[e]
            a1[:, :, g] = (inv1 * c1).reshape(2, 128).T
            b1[:, :, g] = (bb1 * f32(lv - 1)).reshape(2, 128).T
            a2[:, :, g] = (inv2 * c2).reshape(2, 128).T
            b2[:, :, g] = (bb2 * f32(lv - 1)).reshape(2, 128).T
        o = 2 * NG
        cc[:, o:o + 2 * NG] = a1.reshape(128, 2 * NG); o += 2 * NG
        cc[:, o:o + 2 * NG] = b1.reshape(128, 2 * NG); o += 2 * NG
        cc[:, o:o + 2 * NG] = a2.reshape(128, 2 * NG); o += 2 * NG
        cc[:, o:o + 2 * NG] = b2.reshape(128, 2 * NG); o += 2 * NG
        cc[:, o:o + 8] = gn_b.reshape(8, 128).T; o += 8
        cc[:, o:o + 8] = gn_g.reshape(8, 128).T; o += 8

        gr = np.zeros((1, sum(8 * n for n in group_sizes)), f32)
        off = 0
        for g in range(NG):
            ns = group_sizes[g]
            e = experts[g]
            lv = glv[g]
            c3e = CW[e][2] / f32(lv - 1)
            gr[0, off:off + 4 * ns] = c3e
            gr[0, off + 4 * ns:off + 8 * ns] = c3e * c3e
            off += 8 * ns

        in_maps.append({"x": xc, "wp": wpc, "cc": cc, "gr": gr})

    key = group_sizes
    if key not in _NC_CACHE:
        _NC_CACHE[key] = _build_nc(group_sizes)
    nc = _NC_CACHE[key]

    res = run_bass_kernel_spmd(nc, in_maps, core_ids=list(range(NCORES)))

    out = np.zeros((B, OUTC, H, W), f32)
    for c in range(8):
        oc = np.asarray(res.results[c]["out"]).astype(f32)  # [128, 8, 784]
        oc = oc.reshape(128, 8, 4, PIX).transpose(2, 1, 0, 3) \
               .reshape(4, OUTC, H, W)
        for t, sid in enumerate(core_samples[c]):
            out[sid] = oc[t]
    return out


# revision 5
# speedup vs baseline: 3.3999x; 3.3999x over previous
"""Trainium2 Bass kernel for quantized-MoE Bottleneck (nn_Bottleneck_37503654429269).

v5 layout (v4 + trace-driven fixes):
- bf16 integer matmuls; exact round via +-2^23 fp32 trick.
- Host-side expert routing: (3,1) or (2,2) sample groups per core.
- NO gpsimd in the datapath (its tensor_scalar is ~5us/op and it starves
  DVE via the shared SBUF port).
- ACT does Relu-affine work (x scale, bn drains) + the final relu-bias;
  DVE does round + min + bn_stats + the GN-scale stt.
- conv3 recompute: pass1 psum feeds bn_stats only (no SBUF drain); after
  the GN stats chain, pass2 recomputes conv3 into psum and the tail reads
  psum directly. Saves the full S3 drain pass on ACT and 9KB/part SBUF.
- GN tail scalars (per stats-group x sample) broadcast across partitions
  with one K=1 ones-matmul into PSUM; PR/QQ combine them with gng/gnb.
- bf16 output (host converts), batched bn_stats for 2-sample chunks.
"""

import numpy as np

BITS = (2, 4, 8)
EPS = 1e-5
B, C_IN, H, W = 32, 1024, 14, 14
WIDTH, OUTC = 256, 1024
PIX = H * W  # 196
NCORES = 8
RB = float(2.0 ** 23)

_NC_CACHE = {}


# ----------------------------------------------------------------------------
# Device program
# ----------------------------------------------------------------------------

def _build_nc(group_sizes):
    from contextlib import ExitStack
    import concourse.bacc as bacc
    import concourse.mybir as mybir
    import concourse.tile as tile

    F32 = mybir.dt.float32
    BF16 = mybir.dt.bfloat16
    ALU = mybir.AluOpType
    ACT = mybir.ActivationFunctionType

    NG = len(group_sizes)
    NS = sum(group_sizes)
    assert NS == 4
    slot0 = [sum(group_sizes[:g]) for g in range(NG)]
    chunks = {g: [(i, min(2, group_sizes[g] - i))
                  for i in range(0, group_sizes[g], 2)] for g in range(NG)}
    nch = {g: len(chunks[g]) for g in range(NG)}

    nc = bacc.Bacc("TRN2", target_bir_lowering=False, debug=False,
                   num_devices=NCORES)

    # ---- dram tensors
    x_d = nc.dram_tensor("x", [128, 8, 4 * PIX], F32, kind="ExternalInput")
    WCOL = 8 * 256 + 9 * 2 * 256 + 2 * 1024  # 8704
    wp_d = nc.dram_tensor("wp", [NG, 128, WCOL], BF16, kind="ExternalInput")
    # XS[NG] XB[NG] A1[2NG] B1[2NG] A2[2NG] B2[2NG] GNB[8] GNG[8]
    NCC = 10 * NG + 16
    cc_d = nc.dram_tensor("cc", [128, NCC], F32, kind="ExternalInput")
    # per group: c3e[4ns] c3sq[4ns] mdiv[4ns] hrow[4ns]
    GRN = sum(16 * n for n in group_sizes)
    gr_d = nc.dram_tensor("gr", [1, GRN], F32, kind="ExternalInput")
    out_d = nc.dram_tensor("out", [128, 8, 4 * PIX], BF16,
                           kind="ExternalOutput")

    with tile.TileContext(nc) as tc, ExitStack() as ctx:
        res = ctx.enter_context(tc.tile_pool(name="res", bufs=1))
        rot = ctx.enter_context(tc.tile_pool(name="rot", bufs=4))
        srt = ctx.enter_context(tc.tile_pool(name="srt", bufs=2))
        mmp = ctx.enter_context(tc.tile_pool(name="mmp", bufs=6, space="PSUM"))
        smp = ctx.enter_context(tc.tile_pool(name="smp", bufs=2, space="PSUM"))

        # ---- consts + ACT table prewarm (Sqrt/Relu set) before DMAs land
        ONES = res.tile([128, 1], F32, name="ONES", tag="ONES")
        nc.vector.memset(ONES, 1.0)
        ONE1 = res.tile([1, 128], F32, name="ONE1", tag="ONE1")
        nc.vector.memset(ONE1, 1.0)
        WRM = res.tile([1, 2], F32, name="WRM", tag="WRM")
        nc.scalar.activation(out=WRM[:, 0:1], in_=ONE1[:, 0:1], func=ACT.Sqrt,
                             bias=0.0, scale=1.0)
        nc.scalar.activation(out=WRM[:, 1:2], in_=ONE1[:, 0:1], func=ACT.Relu,
                             bias=0.0, scale=1.0)

        # ---- input tiles
        CC = res.tile([128, NCC], F32, name="CC", tag="CC")
        o = 0
        XS = CC[:, o:o + NG]; o += NG
        XB = CC[:, o:o + NG]; o += NG
        A1 = CC[:, o:o + 2 * NG].rearrange("p (m g) -> p m g", m=2); o += 2 * NG
        B1 = CC[:, o:o + 2 * NG].rearrange("p (m g) -> p m g", m=2); o += 2 * NG
        A2 = CC[:, o:o + 2 * NG].rearrange("p (m g) -> p m g", m=2); o += 2 * NG
        B2 = CC[:, o:o + 2 * NG].rearrange("p (m g) -> p m g", m=2); o += 2 * NG
        GNB = CC[:, o:o + 8]; o += 8
        GNG = CC[:, o:o + 8]; o += 8

        GR = res.tile([1, GRN], F32, name="GR", tag="GR")

        XT = res.tile([128, 8, 4 * PIX], F32, name="XT", tag="XT")
        WP = [res.tile([128, WCOL], BF16, name=f"WP{g}", tag=f"WP{g}")
              for g in range(NG)]
        W1 = [WP[g][:, 0:2048].rearrange("p (k m) -> p k m", k=8)
              for g in range(NG)]
        W2 = [WP[g][:, 2048:6656].rearrange("p (t k m) -> p t k m", t=9, k=2)
              for g in range(NG)]
        W3 = [WP[g][:, 6656:8704].rearrange("p (k m) -> p k m", k=2)
              for g in range(NG)]

        # ---- DMA issue order (priority)
        nc.sync.dma_start(out=CC, in_=cc_d.ap())
        nc.sync.dma_start(out=XT[:, 0:2, :], in_=x_d.ap()[:, 0:2, :])
        nc.sync.dma_start(out=WP[0][:, 0:2048], in_=wp_d.ap()[0, :, 0:2048])
        nc.sync.dma_start(out=XT[:, 2:4, :], in_=x_d.ap()[:, 2:4, :])
        nc.sync.dma_start(out=WP[0][:, 2048:], in_=wp_d.ap()[0, :, 2048:])
        nc.sync.dma_start(out=XT[:, 4:6, :], in_=x_d.ap()[:, 4:6, :])
        if NG > 1:
            nc.sync.dma_start(out=WP[1][:, 0:2048],
                              in_=wp_d.ap()[1, :, 0:2048])
        nc.sync.dma_start(out=XT[:, 6:8, :], in_=x_d.ap()[:, 6:8, :])
        if NG > 1:
            nc.sync.dma_start(out=WP[1][:, 2048:], in_=wp_d.ap()[1, :, 2048:])
        nc.sync.dma_start(out=GR, in_=gr_d.ap())

        # ---- persistent stage tiles
        Xq = [res.tile([128, 8, group_sizes[g] * PIX], BF16,
                       name=f"Xq{g}", tag=f"Xq{g}") for g in range(NG)]
        HP = [res.tile([128, 2, group_sizes[g], 16, 18], BF16,
                       name=f"HP{g}", tag=f"HP{g}") for g in range(NG)]
        Q2 = [res.tile([128, 2, group_sizes[g] * PIX], BF16,
                       name=f"Q2{g}", tag=f"Q2{g}") for g in range(NG)]
        OT = [res.tile([128, 8, group_sizes[g] * PIX], BF16,
                       name=f"OT{g}", tag=f"OT{g}") for g in range(NG)]
        BST = [res.tile([128, 8 * nch[g] * 8], F32,
                        name=f"BST{g}", tag=f"BST{g}") for g in range(NG)]
        PR = [res.tile([128, 8, group_sizes[g]], F32,
                       name=f"PR{g}", tag=f"PR{g}") for g in range(NG)]
        QQ = [res.tile([128, 8, group_sizes[g]], F32,
                       name=f"QQ{g}", tag=f"QQ{g}") for g in range(NG)]
        FR = [res.tile([1, 2, 4 * group_sizes[g]], F32,
                       name=f"FR{g}", tag=f"FR{g}") for g in range(NG)]

        # zero only the halo borders of HP (interior is fully overwritten)
        for g in range(NG):
            nc.vector.memset(HP[g][:, :, :, 0:1, :], 0.0)
            nc.vector.memset(HP[g][:, :, :, 15:16, :], 0.0)
            nc.vector.memset(HP[g][:, :, :, 1:15, 0:2], 0.0)
            nc.vector.memset(HP[g][:, :, :, 1:15, 16:18], 0.0)

        # ---------------- x quantization ----------------
        for kt in range(8):
            for g in range(NG):
                ns = group_sizes[g]
                xcols = XT[:, kt, slot0[g] * PIX:(slot0[g] + ns) * PIX]
                u = rot.tile([128, ns * PIX], F32, name="xu", tag=f"xu{g}")
                nc.scalar.activation(out=u, in_=xcols, func=ACT.Relu,
                                     bias=0.0, scale=XS[:, g:g + 1])
                xq = Xq[g][:, kt, :]
                nc.vector.tensor_scalar(out=xq, in0=u, scalar1=RB, scalar2=RB,
                                        op0=ALU.add, op1=ALU.subtract)
                nc.vector.tensor_scalar(out=xq, in0=xq,
                                        scalar1=XB[:, g:g + 1], scalar2=None,
                                        op0=ALU.min)

        # ---------------- conv1 + bn1 + quant ----------------
        def c1_block(g, mo, c0, ncnk):
            ps = mmp.tile([128, ncnk * PIX], F32, name="c1ps", tag="mm")
            for kt in range(8):
                nc.tensor.matmul(
                    ps, W1[g][:, kt, mo * 128:(mo + 1) * 128],
                    Xq[g][:, kt, c0 * PIX:(c0 + ncnk) * PIX],
                    start=(kt == 0), stop=(kt == 7))
            t1 = rot.tile([128, ncnk * PIX], F32, name="t1", tag="t1")
            nc.scalar.activation(out=t1, in_=ps, func=ACT.Relu,
                                 bias=B1[:, mo, g:g + 1],
                                 scale=A1[:, mo, g:g + 1])
            hview = HP[g][:, mo, c0:c0 + ncnk, 1:15, 2:16]
            nc.vector.tensor_scalar(
                out=hview,
                in0=t1.rearrange("p (s y x) -> p s y x", s=ncnk, y=14),
                scalar1=RB, scalar2=RB, op0=ALU.add, op1=ALU.subtract)
            nc.vector.tensor_scalar(out=hview, in0=hview,
                                    scalar1=XB[:, g:g + 1], scalar2=None,
                                    op0=ALU.min)

        for g in range(NG):
            for c0, ncnk in chunks[g]:
                for mo in range(2):
                    c1_block(g, mo, c0, ncnk)

        # ---------------- conv2 + bn2 + quant ----------------
        def c2_block(g, mo, c0, ncnk):
            ps = mmp.tile([128, ncnk, 14, 14], F32, name="c2ps", tag="mm")
            first = True
            for ti, (dy, dx) in enumerate(
                    (dy, dx) for dy in range(3) for dx in range(3)):
                for kt in range(2):
                    nc.tensor.matmul(
                        ps, W2[g][:, ti, kt, mo * 128:(mo + 1) * 128],
                        HP[g][:, kt, c0:c0 + ncnk, dy:dy + 14, dx + 1:dx + 15],
                        start=first, stop=(ti == 8 and kt == 1))
                    first = False
            t2 = rot.tile([128, ncnk * PIX], F32, name="t2", tag="t2")
            nc.scalar.activation(out=t2,
                                 in_=ps.rearrange("p s y x -> p (s y x)"),
                                 func=ACT.Relu, bias=B2[:, mo, g:g + 1],
                                 scale=A2[:, mo, g:g + 1])
            qv = Q2[g][:, mo, c0 * PIX:(c0 + ncnk) * PIX]
            nc.vector.tensor_scalar(out=qv, in0=t2, scalar1=RB, scalar2=RB,
                                    op0=ALU.add, op1=ALU.subtract)
            nc.vector.tensor_scalar(out=qv, in0=qv, scalar1=XB[:, g:g + 1],
                                    scalar2=None, op0=ALU.min)

        # ---------------- conv3 ----------------
        def c3_mm(g, mo, c0, ncnk):
            ps = mmp.tile([128, ncnk * PIX], F32, name="c3ps", tag="mm")
            for kt in range(2):
                nc.tensor.matmul(
                    ps, W3[g][:, kt, mo * 128:(mo + 1) * 128],
                    Q2[g][:, kt, c0 * PIX:(c0 + ncnk) * PIX],
                    start=(kt == 0), stop=(kt == 1))
            return ps

        def c3_pass1(g, mo, ci, c0, ncnk):
            ps = c3_mm(g, mo, c0, ncnk)
            bstv = BST[g][:, 0:8 * nch[g] * 6].rearrange(
                "p (t c) -> p t c", c=6)
            # one bn_stats per chunk: halves are per-sample for ncnk=2
            nc.vector.bn_stats(out=bstv[:, mo * nch[g] + ci:
                                        mo * nch[g] + ci + 1, :],
                               in_=ps)

        def stats_reduce(g):
            nt = 8 * nch[g]
            mvi = BST[g][:, 0:nt * 6].rearrange(
                "p (t h c) -> p t h c", h=2, c=3)[:, :, :, 1]
            msq = BST[g][:, nt * 6:nt * 8].rearrange("p (t h) -> p t h", h=2)
            nc.vector.tensor_tensor(out=msq, in0=mvi, in1=mvi, op=ALU.mult)
            red = mmp.tile([1, nt * 8], F32, name="red", tag="mm")
            nc.tensor.matmul(red, ONES, BST[g], start=True, stop=True)
            return red

        def stats_chain(g, red):
            ns = group_sizes[g]
            nt = 8 * nch[g]
            nsc = 4 * ns
            Tg = srt.tile([1, nt * 8], F32, name="Tg", tag="Tg")
            nc.scalar.activation(out=Tg, in_=red, func=ACT.Copy,
                                 bias=0.0, scale=1.0)
            tv = Tg[:, 0:nt * 6].rearrange("p (m o i c) -> p m o i c",
                                           m=4, o=2, c=6)
            mv = Tg[:, nt * 6:nt * 8].rearrange("p (m o i c) -> p m o i c",
                                                m=4, o=2, c=2)
            TB = srt.tile([1, 4 * nch[g] * 8], F32, name="TB", tag="TB")
            tb6 = TB[:, 0:4 * nch[g] * 6].rearrange(
                "p (m i c) -> p m i c", m=4, c=6)
            tbq = TB[:, 4 * nch[g] * 6:].rearrange(
                "p (m i c) -> p m i c", m=4, c=2)
            nc.vector.tensor_tensor(out=tb6, in0=tv[:, :, 0, :, :],
                                    in1=tv[:, :, 1, :, :], op=ALU.add)
            nc.vector.tensor_tensor(out=tbq, in0=mv[:, :, 0, :, :],
                                    in1=mv[:, :, 1, :, :], op=ALU.add)
            # assemble per-sample rows a (sum mean), b (sum M2), c (sum mean^2)
            AS = srt.tile([1, 3, 4, ns], F32, name="AS", tag="AS")
            for ci, (c0, ncnk) in enumerate(chunks[g]):
                if ncnk == 2:
                    nc.vector.tensor_scalar(
                        out=AS[:, 0, :, c0:c0 + 2], in0=tb6[:, :, ci, 1:6:3],
                        scalar1=0.0, scalar2=None, op0=ALU.add)
                    nc.vector.tensor_scalar(
                        out=AS[:, 1, :, c0:c0 + 2], in0=tb6[:, :, ci, 2:6:3],
                        scalar1=0.0, scalar2=None, op0=ALU.add)
                    nc.vector.tensor_scalar(
                        out=AS[:, 2, :, c0:c0 + 2], in0=tbq[:, :, ci, :],
                        scalar1=0.0, scalar2=None, op0=ALU.add)
                else:
                    nc.vector.tensor_tensor(
                        out=AS[:, 0, :, c0:c0 + 1], in0=tb6[:, :, ci, 1:2],
                        in1=tb6[:, :, ci, 4:5], op=ALU.add)
                    nc.vector.tensor_tensor(
                        out=AS[:, 1, :, c0:c0 + 1], in0=tb6[:, :, ci, 2:3],
                        in1=tb6[:, :, ci, 5:6], op=ALU.add)
                    nc.vector.tensor_tensor(
                        out=AS[:, 2, :, c0:c0 + 1], in0=tbq[:, :, ci, 0:1],
                        in1=tbq[:, :, ci, 1:2], op=ALU.add)
            cbase = sum(16 * n for n in group_sizes[:g])
            C3E = GR[:, cbase:cbase + nsc]
            C3S = GR[:, cbase + nsc:cbase + 2 * nsc]
            MDV = GR[:, cbase + 2 * nsc:cbase + 3 * nsc]
            HRW = GR[:, cbase + 3 * nsc:cbase + 4 * nsc]
            asf = AS.rearrange("p a b c -> p a (b c)")
            MEAN = srt.tile([1, nsc], F32, name="MEAN", tag="MEAN")
            nc.vector.tensor_tensor(out=MEAN, in0=asf[:, 0, :], in1=MDV,
                                    op=ALU.mult)
            E2 = srt.tile([1, nsc], F32, name="E2", tag="E2")
            nc.vector.tensor_tensor(out=E2, in0=asf[:, 2, :], in1=HRW,
                                    op=ALU.mult)
            nc.vector.tensor_tensor(out=E2, in0=E2, in1=asf[:, 1, :],
                                    op=ALU.add)
            nc.vector.tensor_scalar(out=E2, in0=E2,
                                    scalar1=1.0 / (2 * 128 * PIX),
                                    scalar2=None, op0=ALU.mult)
            VAR = srt.tile([1, nsc], F32, name="VAR", tag="VAR")
            nc.vector.tensor_tensor(out=VAR, in0=MEAN, in1=MEAN, op=ALU.mult)
            nc.vector.tensor_tensor(out=VAR, in0=E2, in1=VAR, op=ALU.subtract)
            nc.vector.tensor_tensor(out=VAR, in0=VAR, in1=C3S, op=ALU.mult)
            nc.vector.tensor_scalar(out=VAR, in0=VAR, scalar1=EPS,
                                    scalar2=None, op0=ALU.add)
            SD = srt.tile([1, nsc], F32, name="SD", tag="SD")
            nc.scalar.activation(out=SD, in_=VAR, func=ACT.Sqrt,
                                 bias=0.0, scale=1.0)
            RC = srt.tile([1, nsc], F32, name="RC", tag="RC")
            nc.vector.reciprocal(out=RC, in_=SD)
            nc.vector.tensor_tensor(out=FR[g][:, 0, :], in0=RC, in1=C3E,
                                    op=ALU.mult)
            nc.vector.scalar_tensor_tensor(out=FR[g][:, 1, :], in0=MEAN,
                                           scalar=-1.0, in1=FR[g][:, 0, :],
                                           op0=ALU.mult, op1=ALU.mult)
            rcb = smp.tile([128, 2, nsc], F32, name="rcb", tag="rcb")
            nc.tensor.matmul(rcb.rearrange("p a b -> p (a b)"), ONE1,
                             FR[g].rearrange("p a b -> p (a b)"),
                             start=True, stop=True)
            return rcb

        def prqq_ops(g, rcb):
            ns = group_sizes[g]
            for mo in range(8):
                G3 = mo // 2
                nc.vector.tensor_scalar(
                    out=PR[g][:, mo, :],
                    in0=rcb[:, 0, G3 * ns:(G3 + 1) * ns],
                    scalar1=GNG[:, mo:mo + 1], scalar2=None, op0=ALU.mult)
                nc.vector.tensor_scalar(
                    out=QQ[g][:, mo, :],
                    in0=rcb[:, 1, G3 * ns:(G3 + 1) * ns],
                    scalar1=GNG[:, mo:mo + 1], scalar2=GNB[:, mo:mo + 1],
                    op0=ALU.mult, op1=ALU.add)

        def tail_block(g, mo, c0, ncnk):
            """conv3 pass2 -> stt from psum -> ACT relu-bias -> OT."""
            ps = c3_mm(g, mo, c0, ncnk)
            for si in range(ncnk):
                s = c0 + si
                slot = slot0[g] + s
                V = rot.tile([128, PIX], BF16, name="V", tag="V")
                nc.vector.scalar_tensor_tensor(
                    out=V, in0=ps[:, si * PIX:(si + 1) * PIX],
                    scalar=PR[g][:, mo, s:s + 1],
                    in1=XT[:, mo, slot * PIX:(slot + 1) * PIX],
                    op0=ALU.mult, op1=ALU.add)
                nc.scalar.activation(
                    out=OT[g][:, mo, s * PIX:(s + 1) * PIX], in_=V,
                    func=ACT.Relu, bias=QQ[g][:, mo, s:s + 1], scale=1.0)

        def out_dma(g, mo0, mo1):
            ns = group_sizes[g]
            nc.sync.dma_start(
                out=out_d.ap()[:, mo0:mo1,
                               slot0[g] * PIX:(slot0[g] + ns) * PIX],
                in_=OT[g][:, mo0:mo1, :])

        # ---------------- schedule ----------------
        for c0, ncnk in chunks[0]:
            for mo in range(2):
                c2_block(0, mo, c0, ncnk)
        for mo in range(8):
            for ci, (c0, ncnk) in enumerate(chunks[0]):
                c3_pass1(0, mo, ci, c0, ncnk)
        red0 = stats_reduce(0)

        if NG > 1:
            c2_block(1, 0, chunks[1][0][0], chunks[1][0][1])
        rcb0 = stats_chain(0, red0)
        if NG > 1:
            for c0, ncnk in chunks[1]:
                for mo in range(2):
                    if mo == 0 and c0 == chunks[1][0][0]:
                        continue
                    c2_block(1, mo, c0, ncnk)
        prqq_ops(0, rcb0)
        if NG > 1:
            for mo in range(8):
                for ci, (c0, ncnk) in enumerate(chunks[1]):
                    c3_pass1(1, mo, ci, c0, ncnk)
            red1 = stats_reduce(1)

            # g0 tail (pass2 + stt + relu) overlaps g1 stats chain; keep PE
            # fed with the first two mo's of pass2 while chain1 runs
            for mo in range(2):
                for c0, ncnk in chunks[0]:
                    tail_block(0, mo, c0, ncnk)
            rcb1 = stats_chain(1, red1)
            for mo in range(2, 8):
                for c0, ncnk in chunks[0]:
                    tail_block(0, mo, c0, ncnk)
                if mo == 3:
                    out_dma(0, 0, 4)
            out_dma(0, 4, 8)
            prqq_ops(1, rcb1)
            for mo in range(8):
                for c0, ncnk in chunks[1]:
                    tail_block(1, mo, c0, ncnk)
                if mo == 3:
                    out_dma(1, 0, 4)
            out_dma(1, 4, 8)
        else:
            for mo in range(8):
                for c0, ncnk in chunks[0]:
                    tail_block(0, mo, c0, ncnk)
                if mo == 3:
                    out_dma(0, 0, 4)
            out_dma(0, 4, 8)

    nc.compile()
    return nc


# ----------------------------------------------------------------------------
# Host side
# ----------------------------------------------------------------------------

def _quant_w(w, lv):
    n = max(lv // 2 - 1, 1)
    s = np.float32(np.abs(w).max()) + np.float32(1e-12)
    k = np.round((w.astype(np.float32) / s) * np.float32(n)).astype(np.float32)
    return k, np.float32(s) / np.float32(n)


def _assign_groups(mask):
    mask = np.asarray(mask).astype(np.int64)
    ids = {e: [int(i) for i in np.nonzero(mask == e)[0]] for e in range(3)}
    counts = [len(ids[e]) for e in range(3)]
    if all(c % 2 == 0 for c in counts):
        group_sizes = (2, 2)
        chunks2 = []
        for e in range(3):
            for j in range(0, counts[e], 2):
                chunks2.append((e, ids[e][j:j + 2]))
        assert len(chunks2) == 16
        core_samples = []
        core_experts = []
        for c in range(8):
            (ea, sa), (eb, sb) = chunks2[2 * c], chunks2[2 * c + 1]
            core_samples.append(sa + sb)
            core_experts.append([ea, eb])
        return group_sizes, core_samples, core_experts

    base = [c % 3 for c in counts]
    need = (8 - sum(base)) // 3
    t = [0, 0, 0]
    for e in range(3):
        cap = (counts[e] - base[e]) // 3
        take = min(cap, need)
        t[e] = take
        need -= take
        if need == 0:
            break
    assert need == 0
    b = [base[e] + 3 * t[e] for e in range(3)]
    a = [(counts[e] - b[e]) // 3 for e in range(3)]
    assert sum(a) == 8 and sum(b) == 8
    trip = []
    single = []
    for e in range(3):
        pos = 0
        for _ in range(a[e]):
            trip.append((e, ids[e][pos:pos + 3]))
            pos += 3
        for _ in range(b[e]):
            single.append((e, [ids[e][pos]]))
            pos += 1
        assert pos == counts[e]
    core_samples = []
    core_experts = []
    for c in range(8):
        ea, sa = trip[c]
        eb, sb = single[c]
        core_samples.append(sa + sb)
        core_experts.append([ea, eb])
    return (3, 1), core_samples, core_experts


def kernel(x, mask, w1, w2, w3, bn1_g, bn1_b, bn1_m, bn1_v,
           bn2_g, bn2_b, bn2_m, bn2_v, gn_g, gn_b):
    import ml_dtypes
    from concourse.bass_utils import run_bass_kernel_spmd

    bf16 = ml_dtypes.bfloat16
    f32 = np.float32
    x = np.asarray(x, f32)
    mask = np.asarray(mask)
    w1 = np.asarray(w1, f32)
    w2 = np.asarray(w2, f32)
    w3 = np.asarray(w3, f32)
    bn1 = [np.asarray(v, f32) for v in (bn1_g, bn1_b, bn1_m, bn1_v)]
    bn2 = [np.asarray(v, f32) for v in (bn2_g, bn2_b, bn2_m, bn2_v)]
    gn_g = np.asarray(gn_g, f32)
    gn_b = np.asarray(gn_b, f32)

    group_sizes, core_samples, core_experts = _assign_groups(mask)
    NG = len(group_sizes)
    chunks = {g: [(i, min(2, group_sizes[g] - i))
                  for i in range(0, group_sizes[g], 2)] for g in range(NG)}

    lv_of = [2 ** b for b in BITS]
    K1, K2, K3 = {}, {}, {}
    CW = {}
    for e in range(3):
        lv = lv_of[e]
        k1, c1 = _quant_w(w1, lv)
        k2, c2 = _quant_w(w2, lv)
        k3, c3 = _quant_w(w3, lv)
        K1[e] = k1.reshape(256, 1024)
        K2[e] = k2.reshape(256, 256, 3, 3)
        K3[e] = k3.reshape(1024, 256)
        CW[e] = (c1, c2, c3)

    inv1 = bn1[0] / np.sqrt(bn1[3] + f32(EPS))
    bb1 = bn1[1] - bn1[2] * inv1
    inv2 = bn2[0] / np.sqrt(bn2[3] + f32(EPS))
    bb2 = bn2[1] - bn2[2] * inv2

    def pack_w(e):
        k1t = K1[e].T.reshape(8, 128, 256).transpose(1, 0, 2)
        k2t = K2[e].transpose(2, 3, 1, 0).reshape(9, 2, 128, 256)
        k2t = k2t.transpose(2, 0, 1, 3)
        k3t = K3[e].T.reshape(2, 128, 1024).transpose(1, 0, 2)
        return np.concatenate([
            np.ascontiguousarray(k1t).reshape(128, 2048),
            np.ascontiguousarray(k2t).reshape(128, 4608),
            np.ascontiguousarray(k3t).reshape(128, 2048)], axis=1).astype(bf16)

    packed = {e: pack_w(e) for e in set(int(v) for v in np.asarray(mask))}

    in_maps = []
    for c in range(8):
        sids = core_samples[c]
        experts = core_experts[c]

        xc = x[sids].reshape(4, 8, 128, PIX).transpose(2, 1, 0, 3) \
                    .reshape(128, 8, 4 * PIX).copy()
        wpc = np.stack([packed[experts[g]] for g in range(NG)])

        glv = [lv_of[experts[g]] for g in range(NG)]
        cc = np.zeros((128, 10 * NG + 16), f32)
        cc[:, 0:NG] = [lv - 1 for lv in glv]          # xs
        cc[:, NG:2 * NG] = [lv - 1 for lv in glv]     # xb
        a1 = np.zeros((128, 2, NG), f32)
        b1 = np.zeros((128, 2, NG), f32)
        a2 = np.zeros((128, 2, NG), f32)
        b2 = np.zeros((128, 2, NG), f32)
        for g in range(NG):
            e = experts[g]
            lv = glv[g]
            c1, c2, c3 = CW[e]
            a1[:, :, g] = (inv1 * c1).reshape(2, 128).T
            b1[:, :, g] = (bb1 * f32(lv - 1)).reshape(2, 128).T
            a2[:, :, g] = (inv2 * c2).reshape(2, 128).T
            b2[:, :, g] = (bb2 * f32(lv - 1)).reshape(2, 128).T
        o = 2 * NG
        cc[:, o:o + 2 * NG] = a1.reshape(128, 2 * NG); o += 2 * NG
        cc[:, o:o + 2 * NG] = b1.reshape(128, 2 * NG); o += 2 * NG
        cc[:, o:o + 2 * NG] = a2.reshape(128, 2 * NG); o += 2 * NG
        cc[:, o:o + 2 * NG] = b2.reshape(128, 2 * NG); o += 2 * NG
        cc[:, o:o + 8] = gn_b.reshape(8, 128).T; o += 8
        cc[:, o:o + 8] = gn_g.reshape(8, 128).T; o += 8

        gr = np.zeros((1, sum(16 * n for n in group_sizes)), f32)
        off = 0
        for g in range(NG):
            ns = group_sizes[g]
            e = experts[g]
            lv = glv[g]
            c3e = CW[e][2] / f32(lv - 1)
            # per-sample: 2-chunk samples use 196-halves (mdiv 1/256),
            # 1-chunk samples use 98-halves (mdiv 1/512)
            mdiv = np.zeros(ns, f32)
            hrow = np.zeros(ns, f32)
            for c0, ncnk in chunks[g]:
                for si in range(ncnk):
                    mdiv[c0 + si] = 1.0 / 256 if ncnk == 2 else 1.0 / 512
                    hrow[c0 + si] = 196.0 if ncnk == 2 else 98.0
            gr[0, off:off + 4 * ns] = c3e
            gr[0, off + 4 * ns:off + 8 * ns] = c3e * c3e
            gr[0, off + 8 * ns:off + 12 * ns] = np.tile(mdiv, 4)
            gr[0, off + 12 * ns:off + 16 * ns] = np.tile(hrow, 4)
            off += 16 * ns

        in_maps.append({"x": xc, "wp": wpc, "cc": cc, "gr": gr})

    key = group_sizes
    if key not in _NC_CACHE:
        _NC_CACHE[key] = _build_nc(group_sizes)
    nc = _NC_CACHE[key]

    res = run_bass_kernel_spmd(nc, in_maps, core_ids=list(range(NCORES)))

    out = np.zeros((B, OUTC, H, W), f32)
    for c in range(8):
        oc = np.asarray(res.results[c]["out"]).astype(f32)  # [128, 8, 784]
        oc = oc.reshape(128, 8, 4, PIX).transpose(2, 1, 0, 3) \
               .reshape(4, OUTC, H, W)
        for t, sid in enumerate(core_samples[c]):
            out[sid] = oc[t]
    return out
